# revision 19
# baseline (speedup 1.0000x reference)
"""Batched per-sample video color augmentation (brightness / contrast /
grayscale / hue / identity / saturation) on 8 Trainium2 NeuronCores.

Sharding: by frame (T=8 == 8 cores). Core j processes images[:, :, j, :, :]
for all 8 samples, so the contrast per-frame mean is core-local and the
expensive hue samples spread evenly across cores.

The Bass program is specialized at build time on the values of
selected_augs / hue_factors / blend_factors (tiny per-sample scalars),
which are folded in as immediates; only the image pixels flow through
the kernel.  Identity samples never touch the device (host copy).

Host pre-permutes each sample-frame to [128 partitions, 3*512] so every
DMA line is 6KB contiguous.  Engine placement (measured at [128,512]
f32): DVE tensor_tensor ~0.69us, DVE tensor_scalar (2x_2P) ~0.27us,
ACT ~0.72-0.84us, GpSimd tensor_scalar ~7.4us (avoid for big tiles).
"""

import sys

import numpy as np

if "/opt/trn_rl_repo" not in sys.path:
    sys.path.insert(0, "/opt/trn_rl_repo")

N_SAMPLES = 8
N_CH = 3
N_FRAMES = 8
H = 256
W = 256
HW = H * W          # 65536
P = 128             # SBUF partitions
F = HW // P         # 512 floats per partition per channel
N_CORES = 8

GRAY_R = 0.2989
GRAY_G = 0.587
GRAY_B = 0.114

_PROGRAM_CACHE: dict = {}


def _build_program(augs, hf, bf):
    """augs: list[int] (identity samples excluded by caller's mapping),
    hf/bf: per-sample float32.  The program reads x[NA,128,1536] and
    writes y[NA,128,1536] where NA = number of non-identity samples."""
    import concourse.tile as tile
    from concourse import bacc, bass_isa, mybir

    dt = mybir.dt
    Alu = mybir.AluOpType
    Act = mybir.ActivationFunctionType
    f32 = np.float32

    active = [n for n in range(len(augs)) if int(augs[n]) != 4]
    na = len(active)
    slot = {n: i for i, n in enumerate(active)}

    nc = bacc.Bacc(
        "TRN2", target_bir_lowering=False, debug=False, num_devices=N_CORES
    )
    x = nc.dram_tensor(
        "x", [na, P, N_CH * F], dt.float32, kind="ExternalInput"
    ).ap()
    y = nc.dram_tensor(
        "y", [na, P, N_CH * F], dt.float32, kind="ExternalOutput"
    ).ap()

    c1 = float(f32(GRAY_R) / f32(GRAY_G))
    c2 = float(f32(GRAY_B) / f32(GRAY_G))

    with tile.TileContext(nc) as tc:
        with (
            tc.tile_pool(name="io", bufs=7) as io_pool,
            tc.tile_pool(name="work", bufs=2) as work,
            tc.tile_pool(name="huep", bufs=1) as huep,
            tc.tile_pool(name="small", bufs=2) as small,
            tc.tile_pool(name="consts", bufs=1) as consts,
        ):
            const_tiles: dict = {}

            def cbias(val):
                v = float(f32(val))
                ct = const_tiles.get(v)
                if ct is None:
                    nm = f"cst{len(const_tiles)}"
                    ct = consts.tile([P, 1], dt.float32, name=nm, tag=nm)
                    nc.gpsimd.memset(ct[:], v)
                    const_tiles[v] = ct
                return ct[:]

            def wtile(nm, shape=(P, F)):
                return work.tile(list(shape), dt.float32, name=nm, tag=nm)

            def htile(nm, shape=(P, F)):
                return huep.tile(list(shape), dt.float32, name=nm, tag=nm)

            def stile(nm):
                return small.tile([P, 1], dt.float32, name=nm, tag=nm)

            def yv(n):      # [P, 3, F] view of sample n's output
                return y[slot[n]].rearrange("p (c f) -> p c f", c=N_CH)

            def ych(n, ci):  # [P, F] view of one output channel
                return yv(n)[:, ci]

            hue_n = [n for n in range(len(augs)) if int(augs[n]) == 3]
            contrast_n = [n for n in range(len(augs)) if int(augs[n]) == 1]
            sat_n = [n for n in range(len(augs)) if int(augs[n]) == 5]
            gray_n = [n for n in range(len(augs)) if int(augs[n]) == 2]
            bright_n = [n for n in range(len(augs)) if int(augs[n]) == 0]

            # --- phase 1: loads (hue first: longest dependent chain).
            # Per-channel DMAs so compute can start as soon as the first
            # channels land (subtile deps track per-slice writes). ---
            xts = {}
            for i_n, n in enumerate(hue_n + contrast_n + sat_n + gray_n + bright_n):
                xt = io_pool.tile([P, N_CH, F], dt.float32, name="xt", tag="xt")
                if i_n == 0:
                    # split only the first (hue) load so its DVE chain can
                    # start ~2us earlier; later loads stay whole to keep
                    # sync-ring issue count low
                    for ci in range(N_CH):
                        nc.sync.dma_start(
                            out=xt[:, ci],
                            in_=x[slot[n], :, ci * F:(ci + 1) * F],
                        )
                else:
                    nc.sync.dma_start(out=xt[:], in_=x[slot[n]])
                xts[n] = xt

            # --- phase 2: contrast heads (ACT row-sums -> mean scalar) ---
            contrast_m1 = {}
            for n in contrast_n:
                f = float(f32(bf[n]))
                xt = xts[n]
                scr = wtile("actscr")
                rs = []
                for ci in range(N_CH):
                    rsc = stile(f"rs{ci}")
                    nc.scalar.activation(
                        scr[:], xt[:, ci], Act.Identity,
                        bias=0.0, scale=1.0, accum_out=rsc[:],
                    )
                    rs.append(rsc)
                s1 = stile("s1")
                nc.vector.scalar_tensor_tensor(
                    s1[:], rs[0][:], c1, rs[1][:], Alu.mult, Alu.add
                )
                s2 = stile("s2")
                nc.vector.scalar_tensor_tensor(
                    s2[:], rs[2][:], c2, s1[:], Alu.mult, Alu.add
                )
                tot = stile("tot")
                nc.gpsimd.partition_all_reduce(
                    tot[:], s2[:], channels=P,
                    reduce_op=bass_isa.ReduceOp.add,
                )
                # m1 = (1-f)*mean(gray) = tot * 0.587*(1-f)/65536
                m1 = stile("m1")
                m1_imm = float(f32(f32(GRAY_G) * (f32(1.0) - f32(f)) / f32(HW)))
                nc.gpsimd.tensor_scalar(m1[:], tot[:], m1_imm, None, Alu.mult)
                contrast_m1[n] = m1

            # --- phase 3: hue heads (serial DVE chain through w/q ACTs) ---
            hue_state = {}
            for n in hue_n:
                xt = xts[n]
                r, g, b = xt[:, 0], xt[:, 1], xt[:, 2]
                mx = htile("mx")
                nc.vector.tensor_max(mx[:], r, g)
                v = htile("v")
                nc.vector.tensor_max(v[:], mx[:], b)
                mn = htile("mn")
                nc.vector.tensor_tensor(mn[:], r, g, Alu.min)
                minc = htile("minc")
                nc.vector.tensor_tensor(minc[:], mn[:], b, Alu.min)
                cr = htile("cr")
                nc.vector.tensor_sub(cr[:], v[:], minc[:])
                crd = htile("crd")
                nc.vector.tensor_scalar(crd[:], cr[:], 1e-20, None, Alu.max)
                rcp = htile("rcp")
                rscr = htile("rscr")
                nc.vector.reciprocal_approx_accurate(rcp[:], crd[:], rscr[:])
                er = huep.tile([P, F], dt.uint32, name="er", tag="er")
                nc.vector.tensor_tensor(er[:], v[:], r, Alu.is_equal)
                eg = huep.tile([P, F], dt.uint32, name="eg", tag="eg")
                nc.vector.tensor_tensor(eg[:], v[:], g, Alu.is_equal)
                gb = htile("gb")
                nc.vector.tensor_sub(gb[:], g, b)
                br_ = htile("br_")
                nc.vector.tensor_sub(br_[:], b, r)
                dd = htile("dd")
                nc.vector.tensor_sub(dd[:], r, g)        # b-max case: r-g
                nc.vector.copy_predicated(dd[:], eg[:], br_[:])
                nc.vector.copy_predicated(dd[:], er[:], gb[:])
                dr = htile("dr")
                nc.vector.tensor_mul(dr[:], dd[:], rcp[:])
                z0 = htile("z0")
                nc.vector.tensor_scalar(z0[:], dr[:], 4.0, None, Alu.add)
                t2a = htile("t2a")
                nc.vector.tensor_scalar(t2a[:], dr[:], 2.0, None, Alu.add)
                nc.vector.copy_predicated(z0[:], eg[:], t2a[:])
                nc.vector.copy_predicated(z0[:], er[:], dr[:])
                # per-channel: k2 = wrap6(z0+cc)-2 in one custom op, then
                # |k2| (ACT), relu(|k2|-1) (ACT) -- emitted here so ACT runs
                # them as soon as the wraps land
                hf6 = float(f32(6.0) * f32(hf[n]))
                k_all = htile("k_all", (P, N_CH, F))
                for ci, nconst in enumerate((5.0, 3.0, 1.0)):
                    cc2 = float(f32(f32(nconst) + f32(hf6)) - f32(2.0))
                    nc.vector.add_range_wrap(k_all[:, ci], z0[:], cc2, 4.0, 6.0)
                w_all = htile("w_all", (P, N_CH, F))
                nc.scalar.activation(
                    w_all[:], k_all[:], Act.Abs, bias=0.0, scale=1.0
                )
                q_all = htile("q_all", (P, N_CH, F))
                nc.scalar.activation(
                    q_all[:], w_all[:], Act.Relu, bias=cbias(-1.0), scale=1.0
                )
                hue_state[n] = (q_all, cr, minc)

            # --- phase 4: contrast outs.  f<=1 samples (no clip needed) run
            # on ACT (Identity(x*f + m1)); clip samples use DVE fast TS ---
            for n in contrast_n:
                f = float(f32(bf[n]))
                m1 = contrast_m1[n]
                xt = xts[n]
                yt = io_pool.tile([P, N_CH, F], dt.float32, name="yt", tag="yt")
                if f > 1.0:
                    nc.vector.tensor_scalar(
                        yt[:], xt[:], f, m1[:], Alu.mult, Alu.add
                    )
                    nc.vector.tensor_scalar(
                        yt[:], yt[:], 0.0, 1.0, Alu.max, Alu.min
                    )
                else:
                    nc.scalar.activation(
                        yt[:], xt[:], Act.Identity, bias=m1[:], scale=f
                    )
                nc.sync.dma_start(out=yv(n), in_=yt[:])

            # --- phase 4b: saturation ---
            for n in sat_n:
                f = float(f32(bf[n]))
                xt = xts[n]
                r, g, b = xt[:, 0], xt[:, 1], xt[:, 2]
                t1 = wtile("t1")
                nc.vector.scalar_tensor_tensor(
                    t1[:], r, c1, g, Alu.mult, Alu.add
                )
                t2 = wtile("t2")
                nc.vector.scalar_tensor_tensor(
                    t2[:], b, c2, t1[:], Alu.mult, Alu.add
                )
                g1 = wtile("g1")  # (1-f)*gray
                nc.vector.tensor_scalar(
                    g1[:], t2[:],
                    float(f32(GRAY_G) * (f32(1.0) - f32(f))), None, Alu.mult
                )
                yt = io_pool.tile([P, N_CH, F], dt.float32, name="yt", tag="yt")
                for ci, ch in enumerate((r, g, b)):
                    nc.vector.scalar_tensor_tensor(
                        yt[:, ci], ch, f, g1[:], Alu.mult, Alu.add
                    )
                if f > 1.0:
                    nc.vector.tensor_scalar(
                        yt[:], yt[:], 0.0, 1.0, Alu.max, Alu.min
                    )
                nc.sync.dma_start(out=yv(n), in_=yt[:])

            # --- phase 5: hue tails (qm, pr, out, per-channel stores) ---
            for n in hue_n:
                q_all, cr, minc = hue_state[n]
                qm_all = htile("qm_all", (P, N_CH, F))
                nc.vector.tensor_scalar(
                    qm_all[:], q_all[:], 1.0, None, Alu.min
                )
                for ci in range(N_CH):
                    pr = htile(f"pr{ci}")
                    nc.vector.tensor_mul(pr[:], cr[:], qm_all[:, ci])
                    ho = htile(f"ho{ci}")
                    nc.vector.tensor_add(ho[:], minc[:], pr[:])
                    nc.sync.dma_start(out=ych(n, ci), in_=ho[:])

            # --- phase 5b: gray + brightness ---
            for n in gray_n:
                xt = xts[n]
                r, g, b = xt[:, 0], xt[:, 1], xt[:, 2]
                t1 = wtile("t1")
                nc.vector.scalar_tensor_tensor(
                    t1[:], r, c1, g, Alu.mult, Alu.add
                )
                t2 = wtile("t2")
                nc.vector.scalar_tensor_tensor(
                    t2[:], b, c2, t1[:], Alu.mult, Alu.add
                )
                gray = wtile("gray")
                nc.vector.tensor_scalar(
                    gray[:], t2[:], float(f32(GRAY_G)), None, Alu.mult
                )
                for ci in range(N_CH):
                    nc.sync.dma_start(out=ych(n, ci), in_=gray[:])
            for n in bright_n:
                f = float(f32(bf[n]))
                xt = xts[n]
                yt = io_pool.tile([P, N_CH, F], dt.float32, name="yt", tag="yt")
                nc.vector.tensor_scalar(
                    yt[:], xt[:], f, 1.0, Alu.mult, Alu.min
                )
                nc.sync.dma_start(out=yv(n), in_=yt[:])

    nc.compile()
    return nc


def _get_program(augs, hf, bf):
    key = (tuple(int(v) for v in augs),
           tuple(np.float32(v).tobytes() for v in hf),
           tuple(np.float32(v).tobytes() for v in bf))
    prog = _PROGRAM_CACHE.get(key)
    if prog is None:
        prog = _build_program(augs, hf, bf)
        _PROGRAM_CACHE[key] = prog
    return prog


def _run(images, selected_augs, hue_factors, blend_factors, trace=False):
    from concourse.bass_utils import run_bass_kernel_spmd

    imgs = np.ascontiguousarray(np.asarray(images, dtype=np.float32))
    augs = np.asarray(selected_augs).astype(np.int64)
    hf = np.asarray(hue_factors, dtype=np.float32)
    bf = np.asarray(blend_factors, dtype=np.float32)
    assert imgs.shape == (N_SAMPLES, N_CH, N_FRAMES, H, W), imgs.shape

    active = [n for n in range(N_SAMPLES) if int(augs[n]) != 4]
    out = np.empty((N_SAMPLES, N_CH, N_FRAMES, H, W), dtype=np.float32)
    for n in range(N_SAMPLES):
        if int(augs[n]) == 4:  # identity: out = clip(x) = x, pure copy
            out[n] = imgs[n]

    kres = None
    if active:
        nc = _get_program(augs, hf, bf)
        in_maps = []
        for j in range(N_CORES):
            # [NA, 3, 128, 512] -> [NA, 128, 3, 512]: 6KB-contiguous lines
            xj = imgs[active, :, j].reshape(len(active), N_CH, P, F)
            xj = np.ascontiguousarray(xj.transpose(0, 2, 1, 3)).reshape(
                len(active), P, N_CH * F
            )
            in_maps.append({"x": xj})

        kres = run_bass_kernel_spmd(
            nc, in_maps, list(range(N_CORES)), trace=trace,
            trace_cores=list(range(N_CORES)) if trace else None,
        )

        for j in range(N_CORES):
            yj = kres.results[j]["y"].reshape(len(active), P, N_CH, F)
            yj = yj.transpose(0, 2, 1, 3).reshape(len(active), N_CH, H, W)
            out[active, :, j] = yj
    return out, kres


def kernel(images, selected_augs, hue_factors, blend_factors):
    out, _ = _run(images, selected_augs, hue_factors, blend_factors, trace=False)
    return out


# revision 20
# speedup vs baseline: 1.0516x; 1.0516x over previous
"""Batched per-sample video color augmentation (brightness / contrast /
grayscale / hue / identity / saturation) on 8 Trainium2 NeuronCores.

Sharding: by frame (T=8 == 8 cores). Core j processes images[:, :, j, :, :]
for all 8 samples, so the contrast per-frame mean is core-local and the
expensive hue samples spread evenly across cores.

The Bass program is specialized at build time on the values of
selected_augs / hue_factors / blend_factors (tiny per-sample scalars),
which are folded in as immediates; only the image pixels flow through
the kernel.  Identity samples never touch the device (host copy).

Host pre-permutes each sample-frame to [128 partitions, 3*512] so every
DMA line is 6KB contiguous.  Engine placement (measured at [128,512]
f32): DVE tensor_tensor ~0.69us, DVE tensor_scalar (2x_2P) ~0.27us,
ACT ~0.72-0.84us, GpSimd tensor_scalar ~7.4us (avoid for big tiles).
"""

import sys

import numpy as np

if "/opt/trn_rl_repo" not in sys.path:
    sys.path.insert(0, "/opt/trn_rl_repo")

N_SAMPLES = 8
N_CH = 3
N_FRAMES = 8
H = 256
W = 256
HW = H * W          # 65536
P = 128             # SBUF partitions
F = HW // P         # 512 floats per partition per channel
N_CORES = 8

GRAY_R = 0.2989
GRAY_G = 0.587
GRAY_B = 0.114

_PROGRAM_CACHE: dict = {}


def _build_program(augs, hf, bf):
    """augs: list[int] (identity samples excluded by caller's mapping),
    hf/bf: per-sample float32.  The program reads x[NA,128,1536] and
    writes y[NA,128,1536] where NA = number of non-identity samples."""
    import concourse.tile as tile
    from concourse import bacc, bass_isa, mybir

    dt = mybir.dt
    Alu = mybir.AluOpType
    Act = mybir.ActivationFunctionType
    f32 = np.float32

    active = [n for n in range(len(augs)) if int(augs[n]) != 4]
    na = len(active)
    slot = {n: i for i, n in enumerate(active)}

    nc = bacc.Bacc(
        "TRN2", target_bir_lowering=False, debug=False, num_devices=N_CORES
    )
    x = nc.dram_tensor(
        "x", [na, P, N_CH * F], dt.float32, kind="ExternalInput"
    ).ap()
    y = nc.dram_tensor(
        "y", [na, P, N_CH * F], dt.float32, kind="ExternalOutput"
    ).ap()

    c1 = float(f32(GRAY_R) / f32(GRAY_G))
    c2 = float(f32(GRAY_B) / f32(GRAY_G))

    with tile.TileContext(nc) as tc:
        with (
            tc.tile_pool(name="io", bufs=7) as io_pool,
            tc.tile_pool(name="work", bufs=2) as work,
            tc.tile_pool(name="huep", bufs=1) as huep,
            tc.tile_pool(name="small", bufs=2) as small,
            tc.tile_pool(name="consts", bufs=1) as consts,
        ):
            const_tiles: dict = {}

            def cbias(val):
                v = float(f32(val))
                ct = const_tiles.get(v)
                if ct is None:
                    nm = f"cst{len(const_tiles)}"
                    ct = consts.tile([P, 1], dt.float32, name=nm, tag=nm)
                    nc.gpsimd.memset(ct[:], v)
                    const_tiles[v] = ct
                return ct[:]

            def wtile(nm, shape=(P, F)):
                return work.tile(list(shape), dt.float32, name=nm, tag=nm)

            def htile(nm, shape=(P, F)):
                return huep.tile(list(shape), dt.float32, name=nm, tag=nm)

            def stile(nm):
                return small.tile([P, 1], dt.float32, name=nm, tag=nm)

            def yv(n):      # [P, 3, F] view of sample n's output
                return y[slot[n]].rearrange("p (c f) -> p c f", c=N_CH)

            def ych(n, ci):  # [P, F] view of one output channel
                return yv(n)[:, ci]

            hue_n = [n for n in range(len(augs)) if int(augs[n]) == 3]
            contrast_n = [n for n in range(len(augs)) if int(augs[n]) == 1]
            sat_n = [n for n in range(len(augs)) if int(augs[n]) == 5]
            gray_n = [n for n in range(len(augs)) if int(augs[n]) == 2]
            bright_n = [n for n in range(len(augs)) if int(augs[n]) == 0]

            # --- phase 1: loads (hue first: longest dependent chain).
            # Per-channel DMAs so compute can start as soon as the first
            # channels land (subtile deps track per-slice writes). ---
            xts = {}
            for i_n, n in enumerate(hue_n + contrast_n + sat_n + gray_n + bright_n):
                xt = io_pool.tile([P, N_CH, F], dt.float32, name="xt", tag="xt")
                if i_n == 0:
                    # split only the first (hue) load so its DVE chain can
                    # start ~2us earlier; later loads stay whole to keep
                    # sync-ring issue count low
                    for ci in range(N_CH):
                        nc.sync.dma_start(
                            out=xt[:, ci],
                            in_=x[slot[n], :, ci * F:(ci + 1) * F],
                        )
                else:
                    nc.sync.dma_start(out=xt[:], in_=x[slot[n]])
                xts[n] = xt

            # --- phase 2: contrast heads (ACT row-sums -> mean scalar) ---
            contrast_m1 = {}
            for n in contrast_n:
                f = float(f32(bf[n]))
                xt = xts[n]
                scr = wtile("actscr")
                rs = []
                for ci in range(N_CH):
                    rsc = stile(f"rs{ci}")
                    nc.scalar.activation(
                        scr[:], xt[:, ci], Act.Identity,
                        bias=0.0, scale=1.0, accum_out=rsc[:],
                    )
                    rs.append(rsc)
                s1 = stile("s1")
                nc.vector.scalar_tensor_tensor(
                    s1[:], rs[0][:], c1, rs[1][:], Alu.mult, Alu.add
                )
                s2 = stile("s2")
                nc.vector.scalar_tensor_tensor(
                    s2[:], rs[2][:], c2, s1[:], Alu.mult, Alu.add
                )
                tot = stile("tot")
                nc.gpsimd.partition_all_reduce(
                    tot[:], s2[:], channels=P,
                    reduce_op=bass_isa.ReduceOp.add,
                )
                # m1 = (1-f)*mean(gray) = tot * 0.587*(1-f)/65536
                m1 = stile("m1")
                m1_imm = float(f32(f32(GRAY_G) * (f32(1.0) - f32(f)) / f32(HW)))
                nc.gpsimd.tensor_scalar(m1[:], tot[:], m1_imm, None, Alu.mult)
                contrast_m1[n] = m1

            # --- phase 3: hue heads (serial DVE chain through w/q ACTs) ---
            hue_state = {}
            for n in hue_n:
                xt = xts[n]
                r, g, b = xt[:, 0], xt[:, 1], xt[:, 2]
                mx = htile("mx")
                nc.vector.tensor_max(mx[:], r, g)
                v = htile("v")
                nc.vector.tensor_max(v[:], mx[:], b)
                mn = htile("mn")
                nc.vector.tensor_tensor(mn[:], r, g, Alu.min)
                minc = htile("minc")
                nc.vector.tensor_tensor(minc[:], mn[:], b, Alu.min)
                cr = htile("cr")
                nc.vector.tensor_sub(cr[:], v[:], minc[:])
                crd = htile("crd")
                nc.vector.tensor_scalar(crd[:], cr[:], 1e-20, None, Alu.max)
                rcp = htile("rcp")
                rscr = htile("rscr")
                nc.vector.reciprocal_approx_accurate(rcp[:], crd[:], rscr[:])
                er = huep.tile([P, F], dt.uint32, name="er", tag="er")
                nc.vector.tensor_tensor(er[:], v[:], r, Alu.is_equal)
                eg = huep.tile([P, F], dt.uint32, name="eg", tag="eg")
                nc.vector.tensor_tensor(eg[:], v[:], g, Alu.is_equal)
                gb = htile("gb")
                nc.vector.tensor_sub(gb[:], g, b)
                br_ = htile("br_")
                nc.vector.tensor_sub(br_[:], b, r)
                dd = htile("dd")
                nc.vector.tensor_sub(dd[:], r, g)        # b-max case: r-g
                nc.vector.copy_predicated(dd[:], eg[:], br_[:])
                nc.vector.copy_predicated(dd[:], er[:], gb[:])
                dr = htile("dr")
                nc.vector.tensor_mul(dr[:], dd[:], rcp[:])
                z0 = htile("z0")
                nc.vector.tensor_scalar(z0[:], dr[:], 4.0, None, Alu.add)
                t2a = htile("t2a")
                nc.vector.tensor_scalar(t2a[:], dr[:], 2.0, None, Alu.add)
                nc.vector.copy_predicated(z0[:], eg[:], t2a[:])
                nc.vector.copy_predicated(z0[:], er[:], dr[:])
                # per-channel: k2 = wrap6(z0+cc)-2 in one custom op, then
                # |k2| (ACT), relu(|k2|-1) (ACT) -- emitted here so ACT runs
                # them as soon as the wraps land
                hf6 = float(f32(6.0) * f32(hf[n]))
                k_all = htile("k_all", (P, N_CH, F))
                for ci, nconst in enumerate((5.0, 3.0, 1.0)):
                    cc2 = float(f32(f32(nconst) + f32(hf6)) - f32(2.0))
                    nc.vector.add_range_wrap(k_all[:, ci], z0[:], cc2, 4.0, 6.0)
                w_all = htile("w_all", (P, N_CH, F))
                nc.scalar.activation(
                    w_all[:], k_all[:], Act.Abs, bias=0.0, scale=1.0
                )
                q_all = htile("q_all", (P, N_CH, F))
                nc.scalar.activation(
                    q_all[:], w_all[:], Act.Relu, bias=cbias(-1.0), scale=1.0
                )
                hue_state[n] = (q_all, cr, minc)

            # --- phase 4: contrast outs.  f<=1 samples (no clip needed) run
            # on ACT (Identity(x*f + m1)); clip samples use DVE fast TS ---
            for n in contrast_n:
                f = float(f32(bf[n]))
                m1 = contrast_m1[n]
                xt = xts[n]
                yt = io_pool.tile([P, N_CH, F], dt.float32, name="yt", tag="yt")
                nc.vector.tensor_scalar(
                    yt[:], xt[:], f, m1[:], Alu.mult, Alu.add
                )
                if f > 1.0:
                    nc.vector.tensor_scalar(
                        yt[:], yt[:], 0.0, 1.0, Alu.max, Alu.min
                    )
                nc.sync.dma_start(out=yv(n), in_=yt[:])

            # --- phase 4b: saturation ---
            for n in sat_n:
                f = float(f32(bf[n]))
                xt = xts[n]
                r, g, b = xt[:, 0], xt[:, 1], xt[:, 2]
                t1 = wtile("t1")
                nc.vector.scalar_tensor_tensor(
                    t1[:], r, c1, g, Alu.mult, Alu.add
                )
                t2 = wtile("t2")
                nc.vector.scalar_tensor_tensor(
                    t2[:], b, c2, t1[:], Alu.mult, Alu.add
                )
                g1 = wtile("g1")  # (1-f)*gray
                nc.vector.tensor_scalar(
                    g1[:], t2[:],
                    float(f32(GRAY_G) * (f32(1.0) - f32(f))), None, Alu.mult
                )
                yt = io_pool.tile([P, N_CH, F], dt.float32, name="yt", tag="yt")
                for ci, ch in enumerate((r, g, b)):
                    nc.vector.scalar_tensor_tensor(
                        yt[:, ci], ch, f, g1[:], Alu.mult, Alu.add
                    )
                if f > 1.0:
                    nc.vector.tensor_scalar(
                        yt[:], yt[:], 0.0, 1.0, Alu.max, Alu.min
                    )
                nc.sync.dma_start(out=yv(n), in_=yt[:])

            # --- phase 5: hue tails (qm, pr, out, per-channel stores) ---
            for n in hue_n:
                q_all, cr, minc = hue_state[n]
                qm_all = htile("qm_all", (P, N_CH, F))
                nc.vector.tensor_scalar(
                    qm_all[:], q_all[:], 1.0, None, Alu.min
                )
                for ci in range(N_CH):
                    pr = htile(f"pr{ci}")
                    nc.vector.tensor_mul(pr[:], cr[:], qm_all[:, ci])
                    ho = htile(f"ho{ci}")
                    nc.vector.tensor_add(ho[:], minc[:], pr[:])
                    nc.sync.dma_start(out=ych(n, ci), in_=ho[:])

            # --- phase 5b: gray + brightness ---
            for n in gray_n:
                xt = xts[n]
                r, g, b = xt[:, 0], xt[:, 1], xt[:, 2]
                t1 = wtile("t1")
                nc.vector.scalar_tensor_tensor(
                    t1[:], r, c1, g, Alu.mult, Alu.add
                )
                t2 = wtile("t2")
                nc.vector.scalar_tensor_tensor(
                    t2[:], b, c2, t1[:], Alu.mult, Alu.add
                )
                gray = wtile("gray")
                nc.vector.tensor_scalar(
                    gray[:], t2[:], float(f32(GRAY_G)), None, Alu.mult
                )
                for ci in range(N_CH):
                    nc.sync.dma_start(out=ych(n, ci), in_=gray[:])
            for n in bright_n:
                f = float(f32(bf[n]))
                xt = xts[n]
                yt = io_pool.tile([P, N_CH, F], dt.float32, name="yt", tag="yt")
                nc.vector.tensor_scalar(
                    yt[:], xt[:], f, 1.0, Alu.mult, Alu.min
                )
                nc.sync.dma_start(out=yv(n), in_=yt[:])

    nc.compile()
    return nc


def _get_program(augs, hf, bf):
    key = (tuple(int(v) for v in augs),
           tuple(np.float32(v).tobytes() for v in hf),
           tuple(np.float32(v).tobytes() for v in bf))
    prog = _PROGRAM_CACHE.get(key)
    if prog is None:
        prog = _build_program(augs, hf, bf)
        _PROGRAM_CACHE[key] = prog
    return prog


def _run(images, selected_augs, hue_factors, blend_factors, trace=False):
    from concourse.bass_utils import run_bass_kernel_spmd

    imgs = np.ascontiguousarray(np.asarray(images, dtype=np.float32))
    augs = np.asarray(selected_augs).astype(np.int64)
    hf = np.asarray(hue_factors, dtype=np.float32)
    bf = np.asarray(blend_factors, dtype=np.float32)
    assert imgs.shape == (N_SAMPLES, N_CH, N_FRAMES, H, W), imgs.shape

    active = [n for n in range(N_SAMPLES) if int(augs[n]) != 4]
    out = np.empty((N_SAMPLES, N_CH, N_FRAMES, H, W), dtype=np.float32)
    for n in range(N_SAMPLES):
        if int(augs[n]) == 4:  # identity: out = clip(x) = x, pure copy
            out[n] = imgs[n]

    kres = None
    if active:
        nc = _get_program(augs, hf, bf)
        in_maps = []
        for j in range(N_CORES):
            # [NA, 3, 128, 512] -> [NA, 128, 3, 512]: 6KB-contiguous lines
            xj = imgs[active, :, j].reshape(len(active), N_CH, P, F)
            xj = np.ascontiguousarray(xj.transpose(0, 2, 1, 3)).reshape(
                len(active), P, N_CH * F
            )
            in_maps.append({"x": xj})

        kres = run_bass_kernel_spmd(
            nc, in_maps, list(range(N_CORES)), trace=trace,
            trace_cores=list(range(N_CORES)) if trace else None,
        )

        for j in range(N_CORES):
            yj = kres.results[j]["y"].reshape(len(active), P, N_CH, F)
            yj = yj.transpose(0, 2, 1, 3).reshape(len(active), N_CH, H, W)
            out[active, :, j] = yj
    return out, kres


def kernel(images, selected_augs, hue_factors, blend_factors):
    out, _ = _run(images, selected_augs, hue_factors, blend_factors, trace=False)
    return out


# revision 21
# speedup vs baseline: 1.0828x; 1.0296x over previous
"""Batched per-sample video color augmentation (brightness / contrast /
grayscale / hue / identity / saturation) on 8 Trainium2 NeuronCores.

Sharding: by frame (T=8 == 8 cores). Core j processes images[:, :, j, :, :]
for all 8 samples, so the contrast per-frame mean is core-local and the
expensive hue samples spread evenly across cores.

The Bass program is specialized at build time on the values of
selected_augs / hue_factors / blend_factors (tiny per-sample scalars),
which are folded in as immediates; only the image pixels flow through
the kernel.  Identity samples never touch the device (host copy).

Host pre-permutes each sample-frame to [128 partitions, 3*512] so every
DMA line is 6KB contiguous.  Engine placement (measured at [128,512]
f32): DVE tensor_tensor ~0.69us, DVE tensor_scalar (2x_2P) ~0.27us,
ACT ~0.72-0.84us, GpSimd tensor_scalar ~7.4us (avoid for big tiles).
"""

import sys

import numpy as np

if "/opt/trn_rl_repo" not in sys.path:
    sys.path.insert(0, "/opt/trn_rl_repo")

N_SAMPLES = 8
N_CH = 3
N_FRAMES = 8
H = 256
W = 256
HW = H * W          # 65536
P = 128             # SBUF partitions
F = HW // P         # 512 floats per partition per channel
N_CORES = 8

GRAY_R = 0.2989
GRAY_G = 0.587
GRAY_B = 0.114

_PROGRAM_CACHE: dict = {}


def _build_program(augs, hf, bf):
    """augs: list[int] (identity samples excluded by caller's mapping),
    hf/bf: per-sample float32.  The program reads x[NA,128,1536] and
    writes y[NA,128,1536] where NA = number of non-identity samples."""
    import concourse.tile as tile
    from concourse import bacc, bass_isa, mybir

    dt = mybir.dt
    Alu = mybir.AluOpType
    Act = mybir.ActivationFunctionType
    f32 = np.float32

    active = [n for n in range(len(augs)) if int(augs[n]) != 4]
    na = len(active)
    slot = {n: i for i, n in enumerate(active)}

    nc = bacc.Bacc(
        "TRN2", target_bir_lowering=False, debug=False, num_devices=N_CORES
    )
    x = nc.dram_tensor(
        "x", [na, P, N_CH * F], dt.float32, kind="ExternalInput"
    ).ap()
    y = nc.dram_tensor(
        "y", [na, P, N_CH * F], dt.float32, kind="ExternalOutput"
    ).ap()

    c1 = float(f32(GRAY_R) / f32(GRAY_G))
    c2 = float(f32(GRAY_B) / f32(GRAY_G))

    with tile.TileContext(nc) as tc:
        with (
            tc.tile_pool(name="io", bufs=7) as io_pool,
            tc.tile_pool(name="work", bufs=2) as work,
            tc.tile_pool(name="huep", bufs=1) as huep,
            tc.tile_pool(name="small", bufs=2) as small,
            tc.tile_pool(name="consts", bufs=1) as consts,
        ):
            const_tiles: dict = {}

            def cbias(val):
                v = float(f32(val))
                ct = const_tiles.get(v)
                if ct is None:
                    nm = f"cst{len(const_tiles)}"
                    ct = consts.tile([P, 1], dt.float32, name=nm, tag=nm)
                    nc.gpsimd.memset(ct[:], v)
                    const_tiles[v] = ct
                return ct[:]

            def wtile(nm, shape=(P, F)):
                return work.tile(list(shape), dt.float32, name=nm, tag=nm)

            def htile(nm, shape=(P, F)):
                return huep.tile(list(shape), dt.float32, name=nm, tag=nm)

            def stile(nm):
                return small.tile([P, 1], dt.float32, name=nm, tag=nm)

            def yv(n):      # [P, 3, F] view of sample n's output
                return y[slot[n]].rearrange("p (c f) -> p c f", c=N_CH)

            def ych(n, ci):  # [P, F] view of one output channel
                return yv(n)[:, ci]

            hue_n = [n for n in range(len(augs)) if int(augs[n]) == 3]
            contrast_n = [n for n in range(len(augs)) if int(augs[n]) == 1]
            sat_n = [n for n in range(len(augs)) if int(augs[n]) == 5]
            gray_n = [n for n in range(len(augs)) if int(augs[n]) == 2]
            bright_n = [n for n in range(len(augs)) if int(augs[n]) == 0]

            # --- phase 1: loads (hue first: longest dependent chain).
            # Per-channel DMAs so compute can start as soon as the first
            # channels land (subtile deps track per-slice writes). ---
            xts = {}
            for n in hue_n + contrast_n + sat_n + gray_n + bright_n:
                xt = io_pool.tile([P, N_CH, F], dt.float32, name="xt", tag="xt")
                nc.sync.dma_start(out=xt[:], in_=x[slot[n]])
                xts[n] = xt

            # --- phase 2: contrast heads (ACT row-sums -> mean scalar) ---
            contrast_m1 = {}
            for n in contrast_n:
                f = float(f32(bf[n]))
                xt = xts[n]
                scr = wtile("actscr")
                rs = []
                for ci in range(N_CH):
                    rsc = stile(f"rs{ci}")
                    nc.scalar.activation(
                        scr[:], xt[:, ci], Act.Identity,
                        bias=0.0, scale=1.0, accum_out=rsc[:],
                    )
                    rs.append(rsc)
                s1 = stile("s1")
                nc.vector.scalar_tensor_tensor(
                    s1[:], rs[0][:], c1, rs[1][:], Alu.mult, Alu.add
                )
                s2 = stile("s2")
                nc.vector.scalar_tensor_tensor(
                    s2[:], rs[2][:], c2, s1[:], Alu.mult, Alu.add
                )
                tot = stile("tot")
                nc.gpsimd.partition_all_reduce(
                    tot[:], s2[:], channels=P,
                    reduce_op=bass_isa.ReduceOp.add,
                )
                # m1 = (1-f)*mean(gray) = tot * 0.587*(1-f)/65536
                m1 = stile("m1")
                m1_imm = float(f32(f32(GRAY_G) * (f32(1.0) - f32(f)) / f32(HW)))
                nc.gpsimd.tensor_scalar(m1[:], tot[:], m1_imm, None, Alu.mult)
                contrast_m1[n] = m1

            # --- phase 3: hue heads (serial DVE chain through w/q ACTs) ---
            hue_state = {}
            for n in hue_n:
                xt = xts[n]
                r, g, b = xt[:, 0], xt[:, 1], xt[:, 2]
                mx = htile("mx")
                nc.vector.tensor_max(mx[:], r, g)
                v = htile("v")
                nc.vector.tensor_max(v[:], mx[:], b)
                mn = htile("mn")
                nc.vector.tensor_tensor(mn[:], r, g, Alu.min)
                minc = htile("minc")
                nc.vector.tensor_tensor(minc[:], mn[:], b, Alu.min)
                cr = htile("cr")
                nc.vector.tensor_sub(cr[:], v[:], minc[:])
                crd = htile("crd")
                nc.vector.tensor_scalar(crd[:], cr[:], 1e-20, None, Alu.max)
                rcp = htile("rcp")
                rscr = htile("rscr")
                nc.vector.reciprocal_approx_accurate(rcp[:], crd[:], rscr[:])
                er = huep.tile([P, F], dt.uint32, name="er", tag="er")
                nc.vector.tensor_tensor(er[:], v[:], r, Alu.is_equal)
                eg = huep.tile([P, F], dt.uint32, name="eg", tag="eg")
                nc.vector.tensor_tensor(eg[:], v[:], g, Alu.is_equal)
                gb = htile("gb")
                nc.vector.tensor_sub(gb[:], g, b)
                br_ = htile("br_")
                nc.vector.tensor_sub(br_[:], b, r)
                dd = htile("dd")
                nc.vector.tensor_sub(dd[:], r, g)        # b-max case: r-g
                nc.vector.copy_predicated(dd[:], eg[:], br_[:])
                nc.vector.copy_predicated(dd[:], er[:], gb[:])
                dr = htile("dr")
                nc.vector.tensor_mul(dr[:], dd[:], rcp[:])
                z0 = htile("z0")
                nc.vector.tensor_scalar(z0[:], dr[:], 4.0, None, Alu.add)
                t2a = htile("t2a")
                nc.vector.tensor_scalar(t2a[:], dr[:], 2.0, None, Alu.add)
                nc.vector.copy_predicated(z0[:], eg[:], t2a[:])
                nc.vector.copy_predicated(z0[:], er[:], dr[:])
                # per-channel: k2 = wrap6(z0+cc)-2 in one custom op, then
                # |k2| (ACT), relu(|k2|-1) (ACT) -- emitted here so ACT runs
                # them as soon as the wraps land
                hf6 = float(f32(6.0) * f32(hf[n]))
                k_all = htile("k_all", (P, N_CH, F))
                for ci, nconst in enumerate((5.0, 3.0, 1.0)):
                    cc2 = float(f32(f32(nconst) + f32(hf6)) - f32(2.0))
                    nc.vector.add_range_wrap(k_all[:, ci], z0[:], cc2, 4.0, 6.0)
                w_all = htile("w_all", (P, N_CH, F))
                nc.scalar.activation(
                    w_all[:], k_all[:], Act.Abs, bias=0.0, scale=1.0
                )
                q_all = htile("q_all", (P, N_CH, F))
                nc.scalar.activation(
                    q_all[:], w_all[:], Act.Relu, bias=cbias(-1.0), scale=1.0
                )
                hue_state[n] = (q_all, cr, minc)

            # --- phase 4: contrast outs.  f<=1 samples (no clip needed) run
            # on ACT (Identity(x*f + m1)); clip samples use DVE fast TS ---
            for n in contrast_n:
                f = float(f32(bf[n]))
                m1 = contrast_m1[n]
                xt = xts[n]
                yt = io_pool.tile([P, N_CH, F], dt.float32, name="yt", tag="yt")
                nc.vector.tensor_scalar(
                    yt[:], xt[:], f, m1[:], Alu.mult, Alu.add
                )
                if f > 1.0:
                    nc.vector.tensor_scalar(
                        yt[:], yt[:], 0.0, 1.0, Alu.max, Alu.min
                    )
                nc.sync.dma_start(out=yv(n), in_=yt[:])

            # --- phase 4b: saturation ---
            for n in sat_n:
                f = float(f32(bf[n]))
                xt = xts[n]
                r, g, b = xt[:, 0], xt[:, 1], xt[:, 2]
                t1 = wtile("t1")
                nc.vector.scalar_tensor_tensor(
                    t1[:], r, c1, g, Alu.mult, Alu.add
                )
                t2 = wtile("t2")
                nc.vector.scalar_tensor_tensor(
                    t2[:], b, c2, t1[:], Alu.mult, Alu.add
                )
                g1 = wtile("g1")  # (1-f)*gray
                nc.vector.tensor_scalar(
                    g1[:], t2[:],
                    float(f32(GRAY_G) * (f32(1.0) - f32(f))), None, Alu.mult
                )
                yt = io_pool.tile([P, N_CH, F], dt.float32, name="yt", tag="yt")
                for ci, ch in enumerate((r, g, b)):
                    nc.vector.scalar_tensor_tensor(
                        yt[:, ci], ch, f, g1[:], Alu.mult, Alu.add
                    )
                if f > 1.0:
                    nc.vector.tensor_scalar(
                        yt[:], yt[:], 0.0, 1.0, Alu.max, Alu.min
                    )
                nc.sync.dma_start(out=yv(n), in_=yt[:])

            # --- phase 5: hue tails (qm, pr, out, per-channel stores) ---
            for n in hue_n:
                q_all, cr, minc = hue_state[n]
                qm_all = htile("qm_all", (P, N_CH, F))
                nc.vector.tensor_scalar(
                    qm_all[:], q_all[:], 1.0, None, Alu.min
                )
                for ci in range(N_CH):
                    pr = htile(f"pr{ci}")
                    nc.vector.tensor_mul(pr[:], cr[:], qm_all[:, ci])
                    ho = htile(f"ho{ci}")
                    nc.vector.tensor_add(ho[:], minc[:], pr[:])
                    nc.sync.dma_start(out=ych(n, ci), in_=ho[:])

            # --- phase 5b: gray + brightness ---
            for n in gray_n:
                xt = xts[n]
                r, g, b = xt[:, 0], xt[:, 1], xt[:, 2]
                t1 = wtile("t1")
                nc.vector.scalar_tensor_tensor(
                    t1[:], r, c1, g, Alu.mult, Alu.add
                )
                t2 = wtile("t2")
                nc.vector.scalar_tensor_tensor(
                    t2[:], b, c2, t1[:], Alu.mult, Alu.add
                )
                gray = wtile("gray")
                nc.vector.tensor_scalar(
                    gray[:], t2[:], float(f32(GRAY_G)), None, Alu.mult
                )
                for ci in range(N_CH):
                    nc.sync.dma_start(out=ych(n, ci), in_=gray[:])
            for n in bright_n:
                f = float(f32(bf[n]))
                xt = xts[n]
                yt = io_pool.tile([P, N_CH, F], dt.float32, name="yt", tag="yt")
                nc.vector.tensor_scalar(
                    yt[:], xt[:], f, 1.0, Alu.mult, Alu.min
                )
                nc.sync.dma_start(out=yv(n), in_=yt[:])

    nc.compile()
    return nc


def _get_program(augs, hf, bf):
    key = (tuple(int(v) for v in augs),
           tuple(np.float32(v).tobytes() for v in hf),
           tuple(np.float32(v).tobytes() for v in bf))
    prog = _PROGRAM_CACHE.get(key)
    if prog is None:
        prog = _build_program(augs, hf, bf)
        _PROGRAM_CACHE[key] = prog
    return prog


def _run(images, selected_augs, hue_factors, blend_factors, trace=False):
    from concourse.bass_utils import run_bass_kernel_spmd

    imgs = np.ascontiguousarray(np.asarray(images, dtype=np.float32))
    augs = np.asarray(selected_augs).astype(np.int64)
    hf = np.asarray(hue_factors, dtype=np.float32)
    bf = np.asarray(blend_factors, dtype=np.float32)
    assert imgs.shape == (N_SAMPLES, N_CH, N_FRAMES, H, W), imgs.shape

    active = [n for n in range(N_SAMPLES) if int(augs[n]) != 4]
    out = np.empty((N_SAMPLES, N_CH, N_FRAMES, H, W), dtype=np.float32)
    for n in range(N_SAMPLES):
        if int(augs[n]) == 4:  # identity: out = clip(x) = x, pure copy
            out[n] = imgs[n]

    kres = None
    if active:
        nc = _get_program(augs, hf, bf)
        in_maps = []
        for j in range(N_CORES):
            # [NA, 3, 128, 512] -> [NA, 128, 3, 512]: 6KB-contiguous lines
            xj = imgs[active, :, j].reshape(len(active), N_CH, P, F)
            xj = np.ascontiguousarray(xj.transpose(0, 2, 1, 3)).reshape(
                len(active), P, N_CH * F
            )
            in_maps.append({"x": xj})

        kres = run_bass_kernel_spmd(
            nc, in_maps, list(range(N_CORES)), trace=trace,
            trace_cores=list(range(N_CORES)) if trace else None,
        )

        for j in range(N_CORES):
            yj = kres.results[j]["y"].reshape(len(active), P, N_CH, F)
            yj = yj.transpose(0, 2, 1, 3).reshape(len(active), N_CH, H, W)
            out[active, :, j] = yj
    return out, kres


def kernel(images, selected_augs, hue_factors, blend_factors):
    out, _ = _run(images, selected_augs, hue_factors, blend_factors, trace=False)
    return out
